# revision 33
# baseline (speedup 1.0000x reference)
"""RNN-T JointNetwork kernel for 8 Trainium2 NeuronCores.

Math: out[b,t,u,:] = tanh(concat(fe[b,t], gd[b,u])) @ Wj + bj
with fe = f@We+be, gd = g@Wd+bd.

Since tanh acts elementwise and the concat feeds a single GEMM, the joint
GEMM factorizes exactly:
    out[b,t,u,:] = A[b,t,:] + C[b,u,:]
    A = tanh(f@We+be) @ Wj[:Dm]          (per-(b,t) row)
    C = tanh(g@Wd+bd) @ Wj[Dm:] + bj     (per-(b,u) row)
This collapses the 137-GFLOP joint GEMM into two tiny GEMMs plus a
broadcast-add, leaving the kernel bound by the 268 MB output write
(~90 us/core at 358 GB/s HBM).

Sharding: 8 cores, core c owns (b = c//2, t-half = c%2) -> a [128,64,V]
output chunk per core (contiguous 33.5 MB).

On-core plan: weights/activations are pre-cast to bf16 on the host
(tolerance is 2e-2; bf16 costs ~3e-3), halving weight-load bytes and
running every GEMM at bf16 rate.  C-path loads (g, Wd, Wj-bottom) ride
the sync HWDGE ring, A-path (f, We, Wj-top) the scalar ring, so both
dependency chains stream in parallel.  Prologue:
  - fT/gT via PE transpose, fe/gd GEMMs, tanh (+bias) -> tfT/tgT (bf16)
  - Cp[u,v] = tgT.T@Wj_bot + bj (PSUM) -> bf16; Crep = selrep-stacked Cp
    in fp32 [128,V]
  - A[t,v] = tfT.T@Wj_top (PSUM) -> A_bf bf16 [128,V]
Main loop, per 128-row output tile k (t-pair 2k,2k+1):
  - psO[:,vs] = sel32-slice.T @ A_bf[32q:32q+32, vs]  (K=32 row-broadcast)
  - out_sb = psO + Crep on DVE (fused PSUM->SBUF move)
  - 512 KB contiguous DMA per tile on the sync ring
"""

import sys

sys.path.insert(0, "/opt/trn_rl_repo")

import ml_dtypes
import numpy as np

import concourse.bacc as bacc
import concourse.mybir as mybir
import concourse.tile as tile
from concourse.bass_utils import run_bass_kernel_spmd

B, T, U = 4, 256, 64
D = 512  # DE = DD = DM
V = 1024
TC = 128  # t rows per core
NCORES = 8
FP32 = mybir.dt.float32
BF16 = mybir.dt.bfloat16
TANH = mybir.ActivationFunctionType.Tanh
NPBF16 = ml_dtypes.bfloat16

_cache = {}


def _build_nc():
    nc = bacc.Bacc("TRN2", target_bir_lowering=False)

    fT_d = nc.dram_tensor("fT_c", [D, TC], BF16, kind="ExternalInput")
    gT_d = nc.dram_tensor("gT_c", [D, U], BF16, kind="ExternalInput")
    We_d = nc.dram_tensor("We", [D, D], BF16, kind="ExternalInput")
    Wd_d = nc.dram_tensor("Wd", [D, D], BF16, kind="ExternalInput")
    Wj_d = nc.dram_tensor("Wj", [2 * D, V], BF16, kind="ExternalInput")
    bp_d = nc.dram_tensor("bias_pack", [128, 8], FP32, kind="ExternalInput")
    bj_d = nc.dram_tensor("bj", [1, V], BF16, kind="ExternalInput")
    out_d = nc.dram_tensor("out", [TC * U, V], FP32, kind="ExternalOutput")

    with tile.TileContext(nc) as tc:
        with (
            tc.tile_pool(name="const", bufs=1) as cp,
            tc.tile_pool(name="wts", bufs=1) as wp,
        ):
            # ---- persistent operands ----
            We_sb = wp.tile([128, 4 * D], BF16, tag="We")
            Wd_sb = wp.tile([128, 4 * D], BF16, tag="Wd")
            WjT_sb = wp.tile([128, 4 * V], BF16, tag="WjT")
            WjB_sb = wp.tile([128, 4 * V], BF16, tag="WjB")
            bp_sb = wp.tile([128, 8], FP32, tag="bp")
            bj_sb = wp.tile([1, V], BF16, tag="bj")
            fT_sb = wp.tile([128, 4 * TC], BF16, tag="fT")
            gT_sb = wp.tile([128, 4 * U], BF16, tag="gT")
            tfT = [wp.tile([128, TC], BF16, tag=f"tfT{c}", name=f"tfT{c}") for c in range(4)]
            # tgT2 holds tanh(gd)^T twice side-by-side so the C GEMM emits
            # the u-replicated [128, V] matrix (Crep) directly
            tgT2 = [wp.tile([128, 128], BF16, tag=f"tgT{c}", name=f"tgT{c}") for c in range(4)]
            A_bf = wp.tile([TC, V], BF16, tag="A")
            Crep = wp.tile([128, V], FP32, tag="Crep")

            # ---- loads first.  Weights ride the sync ring as 6 large DMAs
            # (chunks packed side-by-side in one SBUF tile; MMs slice
            # columns).  f/g arrive pre-transposed via XBAR DMA-transpose on
            # the otherwise-idle scalar ring, straight from DRAM.
            nc.sync.dma_start(
                fT_sb[:].rearrange("p (c t) -> p c t", c=4),
                fT_d.rearrange("(c p) t -> p c t", p=128),
            )
            nc.sync.dma_start(
                gT_sb[:].rearrange("p (c u) -> p c u", c=4),
                gT_d.rearrange("(c p) u -> p c u", p=128),
            )
            nc.sync.dma_start(
                We_sb[:].rearrange("p (c d) -> p c d", c=4),
                We_d.rearrange("(c p) d -> p c d", p=128),
            )
            nc.sync.dma_start(
                Wd_sb[:].rearrange("p (c d) -> p c d", c=4),
                Wd_d.rearrange("(c p) d -> p c d", p=128),
            )
            nc.sync.dma_start(bp_sb[:], bp_d[:])
            nc.sync.dma_start(bj_sb[:], bj_d[:])
            nc.sync.dma_start(
                WjB_sb[:].rearrange("p (c v) -> p c v", c=4),
                Wj_d[512:1024, :].rearrange("(c p) v -> p c v", p=128),
            )
            # WjT rides the scalar ring so both Wj halves stream in parallel
            nc.scalar.dma_start(
                WjT_sb[:].rearrange("p (c v) -> p c v", c=4),
                Wj_d[0:512, :].rearrange("(c p) v -> p c v", p=128),
            )

            # ---- constants ----
            # tiny tanh right away so the ACT table load (~1.3us) happens
            # during the DMA phase, not on the tanh critical path
            warm = cp.tile([1, 1], FP32, tag="warm")
            nc.gpsimd.memset(warm[:], 0.0)
            nc.scalar.activation(warm[:], warm[:], TANH)

            # sel32[32q + t', 128i + 64jh + jl] = 1 iff t' == 2i + jh
            # (identical pattern in each 32-partition strip q); used as
            # [32, 128] slices against 32-row strips of A_bf (K=32).
            sel32 = cp.tile([128, 16 * 128], BF16, tag="sel32")
            nc.gpsimd.memset(sel32[:], 0.0)
            for q in range(4):
                sl = sel32[32 * q : 32 * q + 32, :]
                nc.gpsimd.affine_select(
                    out=sl.rearrange("p (i a b) -> p i a b", i=16, a=2),
                    in_=sl.rearrange("p (i a b) -> p i a b", i=16, a=2),
                    compare_op=mybir.AluOpType.not_equal,
                    fill=1.0,
                    base=0,
                    pattern=[[-2, 16], [-1, 2], [0, 64]],
                    channel_multiplier=1,
                )

            ones1 = cp.tile([1, 128], BF16, tag="ones1")
            nc.gpsimd.memset(ones1[:], 1.0)

            # ---- prologue (ordered to match DMA arrivals: f/We then Wd
            # then Wj-top then Wj-bot) ----
            # per-tag PSUM rings: fe 4 + big 4 = 8 banks; A/C tiles get
            # their own ring so they never wait on fe/gd releases
            with tc.tile_pool(name="pp", bufs=1, space="PSUM") as pp:
                for mc in range(4):
                    ps = pp.tile([128, TC], FP32, tag="fe", bufs=4)
                    for dc in range(4):
                        nc.tensor.matmul(
                            ps[:],
                            We_sb[:, dc * 512 + mc * 128 : dc * 512 + (mc + 1) * 128],
                            fT_sb[:, dc * TC : (dc + 1) * TC],
                            start=(dc == 0),
                            stop=(dc == 3),
                        )
                    nc.scalar.activation(
                        tfT[mc][:], ps[:], TANH, bias=bp_sb[:, mc : mc + 1]
                    )
                for mc in range(4):
                    ps = pp.tile([128, TC], FP32, tag="fe", bufs=4)
                    for dc in range(4):
                        nc.tensor.matmul(
                            ps[0:128, 0:U],
                            Wd_sb[:, dc * 512 + mc * 128 : dc * 512 + (mc + 1) * 128],
                            gT_sb[:, dc * U : (dc + 1) * U],
                            start=(dc == 0),
                            stop=(dc == 3),
                        )
                    # tanh twice into both u-blocks so the C GEMM replicates
                    # rows; both read the same PSUM, no DVE involved
                    nc.scalar.activation(
                        tgT2[mc][:, 0:U], ps[0:128, 0:U], TANH,
                        bias=bp_sb[:, 4 + mc : 5 + mc],
                    )
                    nc.scalar.activation(
                        tgT2[mc][:, U:128], ps[0:128, 0:U], TANH,
                        bias=bp_sb[:, 4 + mc : 5 + mc],
                    )

                # C path first (WjB loads before WjT): emits the
                # u-replicated Crep directly (+bj)
                for vh in range(2):
                    vs = slice(vh * 512, (vh + 1) * 512)
                    ps = pp.tile([128, 512], FP32, tag="big", bufs=4)
                    # bias MM first: ones1/bj are tiny early loads, so this
                    # runs long before WjB arrives
                    nc.tensor.matmul(
                        ps[:], ones1[:], bj_sb[:, vs], start=True, stop=False
                    )
                    for mc in range(4):
                        nc.tensor.matmul(
                            ps[:],
                            tgT2[mc][:],
                            WjB_sb[:, mc * V + vh * 512 : mc * V + (vh + 1) * 512],
                            start=False,
                            stop=(mc == 3),
                        )
                    if vh == 0:
                        nc.vector.tensor_copy(Crep[:, vs], ps[:])
                    else:
                        nc.scalar.copy(Crep[:, vs], ps[:])

                # A path
                for vh in range(2):
                    vs = slice(vh * 512, (vh + 1) * 512)
                    ps = pp.tile([128, 512], FP32, tag="big", bufs=4)
                    for mc in range(4):
                        nc.tensor.matmul(
                            ps[:],
                            tfT[mc][:],
                            WjT_sb[:, mc * V + vh * 512 : mc * V + (vh + 1) * 512],
                            start=(mc == 0),
                            stop=(mc == 3),
                        )
                    if vh == 0:
                        nc.vector.tensor_copy(A_bf[:, vs], ps[:])
                    else:
                        nc.scalar.copy(A_bf[:, vs], ps[:])

            # ---- main loop: 64 output tiles of [128, 1024] ----
            with (
                tc.tile_pool(name="po", bufs=4, space="PSUM") as po,
                tc.tile_pool(name="ob", bufs=8) as ob,
            ):
                for k in range(64):
                    q, i = k // 16, k % 16
                    rs = slice(32 * q, 32 * q + 32)
                    psO = po.tile([128, V], FP32, tag="psO")
                    out_sb = ob.tile([128, V], FP32, tag="out")
                    for vh in range(2):
                        vs = slice(vh * 512, (vh + 1) * 512)
                        nc.tensor.matmul(
                            psO[:, vs],
                            sel32[rs, i * 128 : (i + 1) * 128],
                            A_bf[rs, vs],
                            start=True,
                            stop=True,
                            tile_position=(32 * q, 0),
                        )
                    # single full-width DVE add does C + the PSUM->SBUF move
                    nc.vector.tensor_add(out_sb[:], psO[:], Crep[:])
                    nc.sync.dma_start(
                        out_d[k * 128 : (k + 1) * 128, :], out_sb[:]
                    )

    nc.compile()
    return nc


def kernel(f, g, We, be, Wd, bd, Wj, bj):
    if "nc" not in _cache:
        _cache["nc"] = _build_nc()
    nc = _cache["nc"]

    bf = lambda x: np.ascontiguousarray(
        np.asarray(x, dtype=np.float32).astype(NPBF16)
    )
    f_bf, g_bf = bf(f), bf(g)
    be32 = np.asarray(be, np.float32).reshape(4, 128).T
    bd32 = np.asarray(bd, np.float32).reshape(4, 128).T
    bias_pack = np.ascontiguousarray(
        np.concatenate([be32, bd32], axis=1), dtype=np.float32
    )
    shared = {
        "We": bf(We), "Wd": bf(Wd), "Wj": bf(Wj),
        "bias_pack": bias_pack, "bj": bf(bj).reshape(1, V),
    }
    in_maps = []
    for c in range(NCORES):
        b, th = c // 2, c % 2
        in_maps.append(
            {
                "fT_c": np.ascontiguousarray(f_bf[b, th * TC : (th + 1) * TC, :].T),
                "gT_c": np.ascontiguousarray(g_bf[b].T),
                **shared,
            }
        )
    res = run_bass_kernel_spmd(nc, in_maps, list(range(NCORES)))
    kernel._last_results = res

    out = np.empty((B, T, U, V), np.float32)
    for c in range(NCORES):
        b, th = c // 2, c % 2
        out[b, th * TC : (th + 1) * TC] = res.results[c]["out"].reshape(TC, U, V)
    return out


# revision 35
# speedup vs baseline: 1.0627x; 1.0627x over previous
"""RNN-T JointNetwork kernel for 8 Trainium2 NeuronCores.

Math: out[b,t,u,:] = tanh(concat(fe[b,t], gd[b,u])) @ Wj + bj
with fe = f@We+be, gd = g@Wd+bd.

Since tanh acts elementwise and the concat feeds a single GEMM, the joint
GEMM factorizes exactly:
    out[b,t,u,:] = A[b,t,:] + C[b,u,:]
    A = tanh(f@We+be) @ Wj[:Dm]          (per-(b,t) row)
    C = tanh(g@Wd+bd) @ Wj[Dm:] + bj     (per-(b,u) row)
This collapses the 137-GFLOP joint GEMM into two tiny GEMMs plus a
broadcast-add, leaving the kernel bound by the 268 MB output write
(~90 us/core at 358 GB/s HBM).

Sharding: 8 cores, core c owns (b = c//2, t-half = c%2) -> a [128,64,V]
output chunk per core (contiguous 33.5 MB).

On-core plan: weights/activations are pre-cast to bf16 on the host
(tolerance is 2e-2; bf16 costs ~3e-3), halving weight-load bytes and
running every GEMM at bf16 rate.  C-path loads (g, Wd, Wj-bottom) ride
the sync HWDGE ring, A-path (f, We, Wj-top) the scalar ring, so both
dependency chains stream in parallel.  Prologue:
  - fT/gT via PE transpose, fe/gd GEMMs, tanh (+bias) -> tfT/tgT (bf16)
  - Cp[u,v] = tgT.T@Wj_bot + bj (PSUM) -> bf16; Crep = selrep-stacked Cp
    in fp32 [128,V]
  - A[t,v] = tfT.T@Wj_top (PSUM) -> A_bf bf16 [128,V]
Main loop, per 128-row output tile k (t-pair 2k,2k+1):
  - psO[:,vs] = sel32-slice.T @ A_bf[32q:32q+32, vs]  (K=32 row-broadcast)
  - out_sb = psO + Crep on DVE (fused PSUM->SBUF move)
  - 512 KB contiguous DMA per tile on the sync ring
"""

import sys

sys.path.insert(0, "/opt/trn_rl_repo")

import ml_dtypes
import numpy as np

import concourse.bacc as bacc
import concourse.mybir as mybir
import concourse.tile as tile
from concourse.bass_utils import run_bass_kernel_spmd

B, T, U = 4, 256, 64
D = 512  # DE = DD = DM
V = 1024
TC = 128  # t rows per core
NCORES = 8
FP32 = mybir.dt.float32
BF16 = mybir.dt.bfloat16
TANH = mybir.ActivationFunctionType.Tanh
NPBF16 = ml_dtypes.bfloat16

_cache = {}


def _build_nc():
    nc = bacc.Bacc("TRN2", target_bir_lowering=False)

    fT_d = nc.dram_tensor("fT_c", [D, TC], BF16, kind="ExternalInput")
    gT_d = nc.dram_tensor("gT_c", [D, U], BF16, kind="ExternalInput")
    We_d = nc.dram_tensor("We", [D, D], BF16, kind="ExternalInput")
    Wd_d = nc.dram_tensor("Wd", [D, D], BF16, kind="ExternalInput")
    Wj_d = nc.dram_tensor("Wj", [2 * D, V], BF16, kind="ExternalInput")
    bp_d = nc.dram_tensor("bias_pack", [128, 8], FP32, kind="ExternalInput")
    bj_d = nc.dram_tensor("bj", [1, V], BF16, kind="ExternalInput")
    out_d = nc.dram_tensor("out", [TC * U, V], FP32, kind="ExternalOutput")

    with tile.TileContext(nc) as tc:
        with (
            tc.tile_pool(name="const", bufs=1) as cp,
            tc.tile_pool(name="wts", bufs=1) as wp,
        ):
            # ---- persistent operands ----
            We_sb = wp.tile([128, 4 * D], BF16, tag="We")
            Wd_sb = wp.tile([128, 4 * D], BF16, tag="Wd")
            WjT_sb = wp.tile([128, 4 * V], BF16, tag="WjT")
            WjB_sb = wp.tile([128, 4 * V], BF16, tag="WjB")
            bp_sb = wp.tile([128, 8], FP32, tag="bp")
            bj_sb = wp.tile([1, V], BF16, tag="bj")
            fT_sb = wp.tile([128, 4 * TC], BF16, tag="fT")
            gT_sb = wp.tile([128, 4 * U], BF16, tag="gT")
            tfT = [wp.tile([128, TC], BF16, tag=f"tfT{c}", name=f"tfT{c}") for c in range(4)]
            # tgT2 holds tanh(gd)^T twice side-by-side so the C GEMM emits
            # the u-replicated [128, V] matrix (Crep) directly
            tgT2 = [wp.tile([128, 128], BF16, tag=f"tgT{c}", name=f"tgT{c}") for c in range(4)]
            A_bf = wp.tile([TC, V], BF16, tag="A")
            Crep = wp.tile([128, V], FP32, tag="Crep")

            # ---- loads first.  Weights ride the sync ring as 6 large DMAs
            # (chunks packed side-by-side in one SBUF tile; MMs slice
            # columns).  f/g arrive pre-transposed via XBAR DMA-transpose on
            # the otherwise-idle scalar ring, straight from DRAM.
            nc.sync.dma_start(
                fT_sb[:].rearrange("p (c t) -> p c t", c=4),
                fT_d.rearrange("(c p) t -> p c t", p=128),
            )
            nc.sync.dma_start(
                gT_sb[:].rearrange("p (c u) -> p c u", c=4),
                gT_d.rearrange("(c p) u -> p c u", p=128),
            )
            nc.sync.dma_start(
                We_sb[:].rearrange("p (c d) -> p c d", c=4),
                We_d.rearrange("(c p) d -> p c d", p=128),
            )
            nc.sync.dma_start(
                Wd_sb[:].rearrange("p (c d) -> p c d", c=4),
                Wd_d.rearrange("(c p) d -> p c d", p=128),
            )
            nc.sync.dma_start(bp_sb[:], bp_d[:])
            nc.sync.dma_start(bj_sb[:], bj_d[:])
            nc.sync.dma_start(
                WjB_sb[:].rearrange("p (c v) -> p c v", c=4),
                Wj_d[512:1024, :].rearrange("(c p) v -> p c v", p=128),
            )
            # WjT rides the scalar ring so both Wj halves stream in parallel
            nc.scalar.dma_start(
                WjT_sb[:].rearrange("p (c v) -> p c v", c=4),
                Wj_d[0:512, :].rearrange("(c p) v -> p c v", p=128),
            )

            # ---- constants ----
            # tiny tanh right away so the ACT table load (~1.3us) happens
            # during the DMA phase, not on the tanh critical path
            warm = cp.tile([1, 1], FP32, tag="warm")
            nc.gpsimd.memset(warm[:], 0.0)
            nc.scalar.activation(warm[:], warm[:], TANH)

            # sel32[32q + t', 128i + 64jh + jl] = 1 iff t' == 2i + jh
            # (identical pattern in each 32-partition strip q); used as
            # [32, 128] slices against 32-row strips of A_bf (K=32).
            sel32 = cp.tile([128, 16 * 128], BF16, tag="sel32")
            nc.gpsimd.memset(sel32[:], 0.0)
            for q in range(4):
                sl = sel32[32 * q : 32 * q + 32, :]
                nc.gpsimd.affine_select(
                    out=sl.rearrange("p (i a b) -> p i a b", i=16, a=2),
                    in_=sl.rearrange("p (i a b) -> p i a b", i=16, a=2),
                    compare_op=mybir.AluOpType.not_equal,
                    fill=1.0,
                    base=0,
                    pattern=[[-2, 16], [-1, 2], [0, 64]],
                    channel_multiplier=1,
                )

            ones1 = cp.tile([1, 128], BF16, tag="ones1")
            nc.gpsimd.memset(ones1[:], 1.0)

            # ---- prologue (ordered to match DMA arrivals: f/We then Wd
            # then Wj-top then Wj-bot) ----
            # per-tag PSUM rings: fe 4 + big 4 = 8 banks; A/C tiles get
            # their own ring so they never wait on fe/gd releases
            with tc.tile_pool(name="pp", bufs=1, space="PSUM") as pp:
                for mc in range(4):
                    ps = pp.tile([128, TC], FP32, tag="fe", bufs=4)
                    for dc in range(4):
                        nc.tensor.matmul(
                            ps[:],
                            We_sb[:, dc * 512 + mc * 128 : dc * 512 + (mc + 1) * 128],
                            fT_sb[:, dc * TC : (dc + 1) * TC],
                            start=(dc == 0),
                            stop=(dc == 3),
                        )
                    nc.scalar.activation(
                        tfT[mc][:], ps[:], TANH, bias=bp_sb[:, mc : mc + 1]
                    )
                for mc in range(4):
                    ps = pp.tile([128, TC], FP32, tag="fe", bufs=4)
                    for dc in range(4):
                        nc.tensor.matmul(
                            ps[0:128, 0:U],
                            Wd_sb[:, dc * 512 + mc * 128 : dc * 512 + (mc + 1) * 128],
                            gT_sb[:, dc * U : (dc + 1) * U],
                            start=(dc == 0),
                            stop=(dc == 3),
                        )
                    # tanh twice into both u-blocks so the C GEMM replicates
                    # rows; both read the same PSUM, no DVE involved
                    nc.scalar.activation(
                        tgT2[mc][:, 0:U], ps[0:128, 0:U], TANH,
                        bias=bp_sb[:, 4 + mc : 5 + mc],
                    )
                    nc.scalar.activation(
                        tgT2[mc][:, U:128], ps[0:128, 0:U], TANH,
                        bias=bp_sb[:, 4 + mc : 5 + mc],
                    )

                # A path first (WjT arrives in parallel on the scalar ring)
                for vh in range(2):
                    vs = slice(vh * 512, (vh + 1) * 512)
                    ps = pp.tile([128, 512], FP32, tag="big", bufs=4)
                    for mc in range(4):
                        nc.tensor.matmul(
                            ps[:],
                            tfT[mc][:],
                            WjT_sb[:, mc * V + vh * 512 : mc * V + (vh + 1) * 512],
                            start=(mc == 0),
                            stop=(mc == 3),
                        )
                    if vh == 0:
                        nc.vector.tensor_copy(A_bf[:, vs], ps[:])
                    else:
                        nc.scalar.copy(A_bf[:, vs], ps[:])

                # C path: emits the u-replicated Crep directly (+bj)
                for vh in range(2):
                    vs = slice(vh * 512, (vh + 1) * 512)
                    ps = pp.tile([128, 512], FP32, tag="big", bufs=4)
                    # bias MM first: ones1/bj are tiny early loads, so this
                    # runs long before WjB arrives
                    nc.tensor.matmul(
                        ps[:], ones1[:], bj_sb[:, vs], start=True, stop=False
                    )
                    for mc in range(4):
                        nc.tensor.matmul(
                            ps[:],
                            tgT2[mc][:],
                            WjB_sb[:, mc * V + vh * 512 : mc * V + (vh + 1) * 512],
                            start=False,
                            stop=(mc == 3),
                        )
                    if vh == 0:
                        nc.vector.tensor_copy(Crep[:, vs], ps[:])
                    else:
                        nc.scalar.copy(Crep[:, vs], ps[:])

            # ---- main loop: 64 output tiles of [128, 1024] ----
            with (
                tc.tile_pool(name="po", bufs=4, space="PSUM") as po,
                tc.tile_pool(name="ob", bufs=8) as ob,
            ):
                for k in range(64):
                    q, i = k // 16, k % 16
                    rs = slice(32 * q, 32 * q + 32)
                    psO = po.tile([128, V], FP32, tag="psO")
                    out_sb = ob.tile([128, V], FP32, tag="out")
                    if k < 4:
                        # first tiles stream per 512-col half so the first
                        # bytes leave right after the C GEMM finishes
                        for vh in range(2):
                            vs = slice(vh * 512, (vh + 1) * 512)
                            nc.tensor.matmul(
                                psO[:, vs],
                                sel32[rs, i * 128 : (i + 1) * 128],
                                A_bf[rs, vs],
                                start=True,
                                stop=True,
                                tile_position=(32 * q, 0),
                            )
                            nc.vector.tensor_add(
                                out_sb[:, vs], psO[:, vs], Crep[:, vs]
                            )
                            nc.sync.dma_start(
                                out_d[k * 128 : (k + 1) * 128, vs], out_sb[:, vs]
                            )
                        continue
                    for vh in range(2):
                        vs = slice(vh * 512, (vh + 1) * 512)
                        nc.tensor.matmul(
                            psO[:, vs],
                            sel32[rs, i * 128 : (i + 1) * 128],
                            A_bf[rs, vs],
                            start=True,
                            stop=True,
                            tile_position=(32 * q, 0),
                        )
                    # single full-width DVE add does C + the PSUM->SBUF move
                    nc.vector.tensor_add(out_sb[:], psO[:], Crep[:])
                    nc.sync.dma_start(
                        out_d[k * 128 : (k + 1) * 128, :], out_sb[:]
                    )

    nc.compile()
    return nc


def kernel(f, g, We, be, Wd, bd, Wj, bj):
    if "nc" not in _cache:
        _cache["nc"] = _build_nc()
    nc = _cache["nc"]

    bf = lambda x: np.ascontiguousarray(
        np.asarray(x, dtype=np.float32).astype(NPBF16)
    )
    f_bf, g_bf = bf(f), bf(g)
    be32 = np.asarray(be, np.float32).reshape(4, 128).T
    bd32 = np.asarray(bd, np.float32).reshape(4, 128).T
    bias_pack = np.ascontiguousarray(
        np.concatenate([be32, bd32], axis=1), dtype=np.float32
    )
    shared = {
        "We": bf(We), "Wd": bf(Wd), "Wj": bf(Wj),
        "bias_pack": bias_pack, "bj": bf(bj).reshape(1, V),
    }
    in_maps = []
    for c in range(NCORES):
        b, th = c // 2, c % 2
        in_maps.append(
            {
                "fT_c": np.ascontiguousarray(f_bf[b, th * TC : (th + 1) * TC, :].T),
                "gT_c": np.ascontiguousarray(g_bf[b].T),
                **shared,
            }
        )
    res = run_bass_kernel_spmd(nc, in_maps, list(range(NCORES)))
    kernel._last_results = res

    out = np.empty((B, T, U, V), np.float32)
    for c in range(NCORES):
        b, th = c // 2, c % 2
        out[b, th * TC : (th + 1) * TC] = res.results[c]["out"].reshape(TC, U, V)
    return out


# revision 41
# speedup vs baseline: 1.1176x; 1.0516x over previous
"""RNN-T JointNetwork kernel for 8 Trainium2 NeuronCores.

Math: out[b,t,u,:] = tanh(concat(fe[b,t], gd[b,u])) @ Wj + bj
with fe = f@We+be, gd = g@Wd+bd.

Since tanh acts elementwise and the concat feeds a single GEMM, the joint
GEMM factorizes exactly:
    out[b,t,u,:] = A[b,t,:] + C[b,u,:]
    A = tanh(f@We+be) @ Wj[:Dm]          (per-(b,t) row)
    C = tanh(g@Wd+bd) @ Wj[Dm:] + bj     (per-(b,u) row)
This collapses the 137-GFLOP joint GEMM into two tiny GEMMs plus a
broadcast-add, leaving the kernel bound by the 268 MB output write
(~90 us/core at 358 GB/s HBM).

Sharding: 8 cores, core c owns (b = c//2, t-half = c%2) -> a [128,64,V]
output chunk per core (contiguous 33.5 MB).

On-core plan: weights/activations are pre-cast to bf16 on the host
(tolerance is 2e-2; bf16 costs ~3e-3), halving weight-load bytes and
running every GEMM at bf16 rate.  C-path loads (g, Wd, Wj-bottom) ride
the sync HWDGE ring, A-path (f, We, Wj-top) the scalar ring, so both
dependency chains stream in parallel.  Prologue:
  - fT/gT via PE transpose, fe/gd GEMMs, tanh (+bias) -> tfT/tgT (bf16)
  - Cp[u,v] = tgT.T@Wj_bot + bj (PSUM) -> bf16; Crep = selrep-stacked Cp
    in fp32 [128,V]
  - A[t,v] = tfT.T@Wj_top (PSUM) -> A_bf bf16 [128,V]
Main loop, per 128-row output tile k (t-pair 2k,2k+1):
  - psO[:,vs] = sel32-slice.T @ A_bf[32q:32q+32, vs]  (K=32 row-broadcast)
  - out_sb = psO + Crep on DVE (fused PSUM->SBUF move)
  - 512 KB contiguous DMA per tile on the sync ring
"""

import sys

sys.path.insert(0, "/opt/trn_rl_repo")

import ml_dtypes
import numpy as np

import concourse.bacc as bacc
import concourse.mybir as mybir
import concourse.tile as tile
from concourse.bass_utils import run_bass_kernel_spmd

B, T, U = 4, 256, 64
D = 512  # DE = DD = DM
V = 1024
TC = 128  # t rows per core
NCORES = 8
FP32 = mybir.dt.float32
BF16 = mybir.dt.bfloat16
TANH = mybir.ActivationFunctionType.Tanh
NPBF16 = ml_dtypes.bfloat16

_cache = {}


def _build_nc():
    nc = bacc.Bacc("TRN2", target_bir_lowering=False)

    fT_d = nc.dram_tensor("fT_c", [D, TC], BF16, kind="ExternalInput")
    gT_d = nc.dram_tensor("gT_c", [D, U], BF16, kind="ExternalInput")
    We_d = nc.dram_tensor("We", [D, D], BF16, kind="ExternalInput")
    Wd_d = nc.dram_tensor("Wd", [D, D], BF16, kind="ExternalInput")
    Wj_d = nc.dram_tensor("Wj", [2 * D, V], BF16, kind="ExternalInput")
    bp_d = nc.dram_tensor("bias_pack", [128, 8], FP32, kind="ExternalInput")
    bj_d = nc.dram_tensor("bj", [1, V], BF16, kind="ExternalInput")
    out_d = nc.dram_tensor("out", [TC * U, V], FP32, kind="ExternalOutput")

    with tile.TileContext(nc) as tc:
        with (
            tc.tile_pool(name="const", bufs=1) as cp,
            tc.tile_pool(name="wts", bufs=1) as wp,
        ):
            # ---- persistent operands ----
            We_sb = wp.tile([128, 4 * D], BF16, tag="We")
            Wd_sb = wp.tile([128, 4 * D], BF16, tag="Wd")
            WjT_sb = wp.tile([128, 4 * V], BF16, tag="WjT")
            WjB_sb = wp.tile([128, 4 * V], BF16, tag="WjB")
            bp_sb = wp.tile([128, 8], FP32, tag="bp")
            bj_sb = wp.tile([1, V], BF16, tag="bj")
            fT_sb = wp.tile([128, 4 * TC], BF16, tag="fT")
            gT_sb = wp.tile([128, 4 * U], BF16, tag="gT")
            tfT = [wp.tile([128, TC], BF16, tag=f"tfT{c}", name=f"tfT{c}") for c in range(4)]
            # tgT2 holds tanh(gd)^T twice side-by-side so the C GEMM emits
            # the u-replicated [128, V] matrix (Crep) directly
            tgT2 = [wp.tile([128, 128], BF16, tag=f"tgT{c}", name=f"tgT{c}") for c in range(4)]
            A_bf = wp.tile([TC, V], BF16, tag="A")
            Crep = wp.tile([128, V], FP32, tag="Crep")

            # ---- loads first.  Weights ride the sync ring as 6 large DMAs
            # (chunks packed side-by-side in one SBUF tile; MMs slice
            # columns).  f/g arrive pre-transposed via XBAR DMA-transpose on
            # the otherwise-idle scalar ring, straight from DRAM.
            nc.sync.dma_start(
                fT_sb[:].rearrange("p (c t) -> p c t", c=4),
                fT_d.rearrange("(c p) t -> p c t", p=128),
            )
            nc.sync.dma_start(
                gT_sb[:].rearrange("p (c u) -> p c u", c=4),
                gT_d.rearrange("(c p) u -> p c u", p=128),
            )
            nc.sync.dma_start(
                We_sb[:].rearrange("p (c d) -> p c d", c=4),
                We_d.rearrange("(c p) d -> p c d", p=128),
            )
            nc.sync.dma_start(
                Wd_sb[:].rearrange("p (c d) -> p c d", c=4),
                Wd_d.rearrange("(c p) d -> p c d", p=128),
            )
            nc.sync.dma_start(bp_sb[:], bp_d[:])
            nc.sync.dma_start(bj_sb[:], bj_d[:])
            nc.sync.dma_start(
                WjB_sb[:].rearrange("p (c v) -> p c v", c=4),
                Wj_d[512:1024, :].rearrange("(c p) v -> p c v", p=128),
            )
            # WjT rides the scalar ring so both Wj halves stream in parallel
            nc.scalar.dma_start(
                WjT_sb[:].rearrange("p (c v) -> p c v", c=4),
                Wj_d[0:512, :].rearrange("(c p) v -> p c v", p=128),
            )

            # ---- constants ----
            # tiny tanh right away so the ACT table load (~1.3us) happens
            # during the DMA phase, not on the tanh critical path
            warm = cp.tile([1, 1], FP32, tag="warm")
            nc.gpsimd.memset(warm[:], 0.0)
            nc.scalar.activation(warm[:], warm[:], TANH)

            # sel32[32q + t', 128i + 64jh + jl] = 1 iff t' == 2i + jh
            # (identical pattern in each 32-partition strip q); used as
            # [32, 128] slices against 32-row strips of A_bf (K=32).
            sel32 = cp.tile([128, 16 * 128], BF16, tag="sel32")
            nc.gpsimd.memset(sel32[:], 0.0)
            for q in range(4):
                sl = sel32[32 * q : 32 * q + 32, :]
                nc.gpsimd.affine_select(
                    out=sl.rearrange("p (i a b) -> p i a b", i=16, a=2),
                    in_=sl.rearrange("p (i a b) -> p i a b", i=16, a=2),
                    compare_op=mybir.AluOpType.not_equal,
                    fill=1.0,
                    base=0,
                    pattern=[[-2, 16], [-1, 2], [0, 64]],
                    channel_multiplier=1,
                )

            ones1 = cp.tile([1, 128], BF16, tag="ones1")
            nc.gpsimd.memset(ones1[:], 1.0)

            # PE warmup source: garbage-free memset tile for dummy matmuls
            # that keep the PE HAM at 8/8 (2.4 GHz) through the load phase
            dummy = cp.tile([128, 512], BF16, tag="dummy")
            nc.gpsimd.memset(dummy[:], 0.0)

            # ---- prologue (ordered to match DMA arrivals: f/We then Wd
            # then Wj-top then Wj-bot) ----
            # per-tag PSUM rings: fe 4 + big 4 = 8 banks; A/C tiles get
            # their own ring so they never wait on fe/gd releases
            with tc.tile_pool(name="pp", bufs=1, space="PSUM") as pp:
                # ---- PE warmup: ~7.5us of back-to-back dummy matmuls so
                # HAM un-throttles the PE clock before the real GEMMs.
                # They chain WAW on one PSUM bank; results are never read.
                psw = pp.tile([128, 512], FP32, tag="big", bufs=4)
                for _ in range(14):
                    nc.tensor.matmul(psw[:], dummy[:, 0:128], dummy[:], start=True, stop=True)

                for mc in range(4):
                    ps = pp.tile([128, TC], FP32, tag="fe", bufs=4)
                    for dc in range(4):
                        nc.tensor.matmul(
                            ps[:],
                            We_sb[:, dc * 512 + mc * 128 : dc * 512 + (mc + 1) * 128],
                            fT_sb[:, dc * TC : (dc + 1) * TC],
                            start=(dc == 0),
                            stop=(dc == 3),
                        )
                    nc.scalar.activation(
                        tfT[mc][:], ps[:], TANH, bias=bp_sb[:, mc : mc + 1]
                    )
                for mc in range(4):
                    ps = pp.tile([128, TC], FP32, tag="fe", bufs=4)
                    for dc in range(4):
                        nc.tensor.matmul(
                            ps[0:128, 0:U],
                            Wd_sb[:, dc * 512 + mc * 128 : dc * 512 + (mc + 1) * 128],
                            gT_sb[:, dc * U : (dc + 1) * U],
                            start=(dc == 0),
                            stop=(dc == 3),
                        )
                    # tanh into the low u-block; DVE (idle here) duplicates
                    # it so the C GEMM replicates rows
                    nc.scalar.activation(
                        tgT2[mc][:, 0:U], ps[0:128, 0:U], TANH,
                        bias=bp_sb[:, 4 + mc : 5 + mc],
                    )
                    nc.vector.tensor_copy(tgT2[mc][:, U:128], tgT2[mc][:, 0:U])

                # A path first (WjT arrives in parallel on the scalar ring)
                for vh in range(2):
                    vs = slice(vh * 512, (vh + 1) * 512)
                    ps = pp.tile([128, 512], FP32, tag="big", bufs=4)
                    for mc in range(4):
                        nc.tensor.matmul(
                            ps[:],
                            tfT[mc][:],
                            WjT_sb[:, mc * V + vh * 512 : mc * V + (vh + 1) * 512],
                            start=(mc == 0),
                            stop=(mc == 3),
                        )
                    if vh == 0:
                        nc.vector.tensor_copy(A_bf[:, vs], ps[:])
                    else:
                        nc.scalar.copy(A_bf[:, vs], ps[:])

                # hoisted first-tile select-MMs (vh0): only need A_bf vh0,
                # so they run while WjB is still streaming in
                psE = [
                    pp.tile([128, 512], FP32, tag="big", bufs=4, name=f"psE{k}")
                    for k in range(2)
                ]
                obE = [
                    wp.tile([128, V], FP32, tag=f"obE{k}", name=f"obE{k}")
                    for k in range(2)
                ]
                for k in range(2):
                    nc.tensor.matmul(
                        psE[k][:],
                        sel32[0:32, k * 128 : (k + 1) * 128],
                        A_bf[0:32, 0:512],
                        start=True,
                        stop=True,
                        tile_position=(0, 0),
                    )

                # C path: emits the u-replicated Crep directly (+bj)
                for vh in range(2):
                    vs = slice(vh * 512, (vh + 1) * 512)
                    ps = pp.tile([128, 512], FP32, tag="big", bufs=4)
                    # bias MM first: ones1/bj are tiny early loads, so this
                    # runs long before WjB arrives
                    nc.tensor.matmul(
                        ps[:], ones1[:], bj_sb[:, vs], start=True, stop=False
                    )
                    for mc in range(4):
                        nc.tensor.matmul(
                            ps[:],
                            tgT2[mc][:],
                            WjB_sb[:, mc * V + vh * 512 : mc * V + (vh + 1) * 512],
                            start=False,
                            stop=(mc == 3),
                        )
                    if vh == 0:
                        nc.vector.tensor_copy(Crep[:, vs], ps[:])
                    else:
                        nc.scalar.copy(Crep[:, vs], ps[:])

                # complete tiles k=0,1: vh0 add+store right after Crep-vh0,
                # then the vh1 half
                for k in range(2):
                    nc.vector.tensor_add(
                        obE[k][:, 0:512], psE[k][:], Crep[:, 0:512]
                    )
                    nc.sync.dma_start(
                        out_d[k * 128 : (k + 1) * 128, 0:512], obE[k][:, 0:512]
                    )
                for k in range(2):
                    psv = pp.tile([128, 512], FP32, tag="big", bufs=4, name=f"psv{k}")
                    nc.tensor.matmul(
                        psv[:],
                        sel32[0:32, k * 128 : (k + 1) * 128],
                        A_bf[0:32, 512:1024],
                        start=True,
                        stop=True,
                        tile_position=(0, 0),
                    )
                    nc.vector.tensor_add(
                        obE[k][:, 512:1024], psv[:], Crep[:, 512:1024]
                    )
                    nc.sync.dma_start(
                        out_d[k * 128 : (k + 1) * 128, 512:1024],
                        obE[k][:, 512:1024],
                    )

            # ---- main loop: remaining 62 output tiles of [128, 1024] ----
            with (
                tc.tile_pool(name="po", bufs=4, space="PSUM") as po,
                tc.tile_pool(name="ob", bufs=8) as ob,
            ):
                for k in range(2, 64):
                    q, i = k // 16, k % 16
                    rs = slice(32 * q, 32 * q + 32)
                    psO = po.tile([128, V], FP32, tag="psO")
                    out_sb = ob.tile([128, V], FP32, tag="out")
                    for vh in range(2):
                        vs = slice(vh * 512, (vh + 1) * 512)
                        nc.tensor.matmul(
                            psO[:, vs],
                            sel32[rs, i * 128 : (i + 1) * 128],
                            A_bf[rs, vs],
                            start=True,
                            stop=True,
                            tile_position=(32 * q, 0),
                        )
                    # single full-width DVE add does C + the PSUM->SBUF move
                    nc.vector.tensor_add(out_sb[:], psO[:], Crep[:])
                    nc.sync.dma_start(
                        out_d[k * 128 : (k + 1) * 128, :], out_sb[:]
                    )

    nc.compile()
    return nc


def kernel(f, g, We, be, Wd, bd, Wj, bj):
    if "nc" not in _cache:
        _cache["nc"] = _build_nc()
    nc = _cache["nc"]

    bf = lambda x: np.ascontiguousarray(
        np.asarray(x, dtype=np.float32).astype(NPBF16)
    )
    f_bf, g_bf = bf(f), bf(g)
    be32 = np.asarray(be, np.float32).reshape(4, 128).T
    bd32 = np.asarray(bd, np.float32).reshape(4, 128).T
    bias_pack = np.ascontiguousarray(
        np.concatenate([be32, bd32], axis=1), dtype=np.float32
    )
    shared = {
        "We": bf(We), "Wd": bf(Wd), "Wj": bf(Wj),
        "bias_pack": bias_pack, "bj": bf(bj).reshape(1, V),
    }
    in_maps = []
    for c in range(NCORES):
        b, th = c // 2, c % 2
        in_maps.append(
            {
                "fT_c": np.ascontiguousarray(f_bf[b, th * TC : (th + 1) * TC, :].T),
                "gT_c": np.ascontiguousarray(g_bf[b].T),
                **shared,
            }
        )
    res = run_bass_kernel_spmd(nc, in_maps, list(range(NCORES)))
    kernel._last_results = res

    out = np.empty((B, T, U, V), np.float32)
    for c in range(NCORES):
        b, th = c // 2, c % 2
        out[b, th * TC : (th + 1) * TC] = res.results[c]["out"].reshape(TC, U, V)
    return out


# revision 44
# speedup vs baseline: 1.1958x; 1.0700x over previous
"""RNN-T JointNetwork kernel for 8 Trainium2 NeuronCores.

Math: out[b,t,u,:] = tanh(concat(fe[b,t], gd[b,u])) @ Wj + bj
with fe = f@We+be, gd = g@Wd+bd.

Since tanh acts elementwise and the concat feeds a single GEMM, the joint
GEMM factorizes exactly:
    out[b,t,u,:] = A[b,t,:] + C[b,u,:]
    A = tanh(f@We+be) @ Wj[:Dm]          (per-(b,t) row)
    C = tanh(g@Wd+bd) @ Wj[Dm:] + bj     (per-(b,u) row)
This collapses the 137-GFLOP joint GEMM into two tiny GEMMs plus a
broadcast-add, leaving the kernel bound by the 268 MB output write
(~90 us/core at 358 GB/s HBM).

Sharding: 8 cores, core c owns (b = c//2, t-half = c%2) -> a [128,64,V]
output chunk per core (contiguous 33.5 MB).

On-core plan: weights/activations are pre-cast to bf16 on the host
(tolerance is 2e-2; bf16 costs ~3e-3), halving weight-load bytes and
running every GEMM at bf16 rate.  C-path loads (g, Wd, Wj-bottom) ride
the sync HWDGE ring, A-path (f, We, Wj-top) the scalar ring, so both
dependency chains stream in parallel.  Prologue:
  - fT/gT via PE transpose, fe/gd GEMMs, tanh (+bias) -> tfT/tgT (bf16)
  - Cp[u,v] = tgT.T@Wj_bot + bj (PSUM) -> bf16; Crep = selrep-stacked Cp
    in fp32 [128,V]
  - A[t,v] = tfT.T@Wj_top (PSUM) -> A_bf bf16 [128,V]
Main loop, per 128-row output tile k (t-pair 2k,2k+1):
  - psO[:,vs] = sel32-slice.T @ A_bf[32q:32q+32, vs]  (K=32 row-broadcast)
  - out_sb = psO + Crep on DVE (fused PSUM->SBUF move)
  - 512 KB contiguous DMA per tile on the sync ring
"""

import sys

sys.path.insert(0, "/opt/trn_rl_repo")

import ml_dtypes
import numpy as np

import concourse.bacc as bacc
import concourse.mybir as mybir
import concourse.tile as tile
from concourse.bass_utils import run_bass_kernel_spmd

B, T, U = 4, 256, 64
D = 512  # DE = DD = DM
V = 1024
TC = 128  # t rows per core
NCORES = 8
FP32 = mybir.dt.float32
BF16 = mybir.dt.bfloat16
TANH = mybir.ActivationFunctionType.Tanh
NPBF16 = ml_dtypes.bfloat16

_cache = {}


def _build_nc():
    nc = bacc.Bacc("TRN2", target_bir_lowering=False)

    fT_d = nc.dram_tensor("fT_c", [D, TC], BF16, kind="ExternalInput")
    gT_d = nc.dram_tensor("gT_c", [D, U], BF16, kind="ExternalInput")
    We_d = nc.dram_tensor("We", [D, D], BF16, kind="ExternalInput")
    Wd_d = nc.dram_tensor("Wd", [D, D], BF16, kind="ExternalInput")
    Wj_d = nc.dram_tensor("Wj", [2 * D, V], BF16, kind="ExternalInput")
    bp_d = nc.dram_tensor("bias_pack", [128, 8], FP32, kind="ExternalInput")
    bj_d = nc.dram_tensor("bj", [1, V], BF16, kind="ExternalInput")
    out_d = nc.dram_tensor("out", [TC * U, V], FP32, kind="ExternalOutput")

    with tile.TileContext(nc) as tc:
        with (
            tc.tile_pool(name="const", bufs=1) as cp,
            tc.tile_pool(name="wts", bufs=1) as wp,
        ):
            # ---- persistent operands ----
            We_sb = wp.tile([128, 4 * D], BF16, tag="We")
            Wd_sb = wp.tile([128, 4 * D], BF16, tag="Wd")
            WjT_sb = wp.tile([128, 4 * V], BF16, tag="WjT")
            WjB_sb = wp.tile([128, 4 * V], BF16, tag="WjB")
            bp_sb = wp.tile([128, 8], FP32, tag="bp")
            bj_sb = wp.tile([1, V], BF16, tag="bj")
            fT_sb = wp.tile([128, 4 * TC], BF16, tag="fT")
            gT_sb = wp.tile([128, 4 * U], BF16, tag="gT")
            tfT = [wp.tile([128, TC], BF16, tag=f"tfT{c}", name=f"tfT{c}") for c in range(4)]
            # tgT2 holds tanh(gd)^T twice side-by-side so the C GEMM emits
            # the u-replicated [128, V] matrix (Crep) directly
            tgT2 = [wp.tile([128, 128], BF16, tag=f"tgT{c}", name=f"tgT{c}") for c in range(4)]
            A_bf = wp.tile([TC, V], BF16, tag="A")
            Crep = wp.tile([128, V], FP32, tag="Crep")

            # ---- loads first.  Weights ride the sync ring as 6 large DMAs
            # (chunks packed side-by-side in one SBUF tile; MMs slice
            # columns).  f/g arrive pre-transposed via XBAR DMA-transpose on
            # the otherwise-idle scalar ring, straight from DRAM.
            nc.sync.dma_start(
                fT_sb[:].rearrange("p (c t) -> p c t", c=4),
                fT_d.rearrange("(c p) t -> p c t", p=128),
            )
            nc.sync.dma_start(
                gT_sb[:].rearrange("p (c u) -> p c u", c=4),
                gT_d.rearrange("(c p) u -> p c u", p=128),
            )
            nc.sync.dma_start(
                We_sb[:].rearrange("p (c d) -> p c d", c=4),
                We_d.rearrange("(c p) d -> p c d", p=128),
            )
            nc.sync.dma_start(
                Wd_sb[:].rearrange("p (c d) -> p c d", c=4),
                Wd_d.rearrange("(c p) d -> p c d", p=128),
            )
            nc.sync.dma_start(bp_sb[:], bp_d[:])
            nc.sync.dma_start(bj_sb[:], bj_d[:])
            nc.sync.dma_start(
                WjB_sb[:].rearrange("p (c v) -> p c v", c=4),
                Wj_d[512:1024, :].rearrange("(c p) v -> p c v", p=128),
            )
            # WjT rides the scalar ring so both Wj halves stream in parallel
            nc.scalar.dma_start(
                WjT_sb[:].rearrange("p (c v) -> p c v", c=4),
                Wj_d[0:512, :].rearrange("(c p) v -> p c v", p=128),
            )

            # ---- constants ----
            # tiny tanh right away so the ACT table load (~1.3us) happens
            # during the DMA phase, not on the tanh critical path
            warm = cp.tile([1, 1], FP32, tag="warm")
            nc.gpsimd.memset(warm[:], 0.0)
            nc.scalar.activation(warm[:], warm[:], TANH)

            # PE warmup source FIRST on gpsimd (sel32's affines take ~11us)
            dummy = cp.tile([128, 512], BF16, tag="dummy")
            nc.gpsimd.memset(dummy[:], 0.0)

            # sel32[32q + t', 128i + 64jh + jl] = 1 iff t' == 2i + jh
            # (identical pattern in each 32-partition strip q); used as
            # [32, 128] slices against 32-row strips of A_bf (K=32).
            sel32 = cp.tile([128, 16 * 128], BF16, tag="sel32")
            nc.gpsimd.memset(sel32[:], 0.0)
            for q in range(4):
                sl = sel32[32 * q : 32 * q + 32, :]
                nc.gpsimd.affine_select(
                    out=sl.rearrange("p (i a b) -> p i a b", i=16, a=2),
                    in_=sl.rearrange("p (i a b) -> p i a b", i=16, a=2),
                    compare_op=mybir.AluOpType.not_equal,
                    fill=1.0,
                    base=0,
                    pattern=[[-2, 16], [-1, 2], [0, 64]],
                    channel_multiplier=1,
                )

            ones1 = cp.tile([1, 128], BF16, tag="ones1")
            nc.gpsimd.memset(ones1[:], 1.0)

            # ---- prologue (ordered to match DMA arrivals: f/We then Wd
            # then Wj-top then Wj-bot) ----
            # per-tag PSUM rings: fe 4 + big 4 = 8 banks; A/C tiles get
            # their own ring so they never wait on fe/gd releases
            with tc.tile_pool(name="pp", bufs=1, space="PSUM") as pp:
                # ---- PE warmup: ~7.5us of back-to-back dummy matmuls so
                # HAM un-throttles the PE clock before the real GEMMs.
                # They chain WAW on one PSUM bank; results are never read.
                psw = pp.tile([128, 512], FP32, tag="big", bufs=4)
                for _ in range(9):
                    nc.tensor.matmul(psw[:], dummy[:, 0:128], dummy[:], start=True, stop=True)

                for mc in range(4):
                    ps = pp.tile([128, TC], FP32, tag="fe", bufs=4)
                    for dc in range(4):
                        nc.tensor.matmul(
                            ps[:],
                            We_sb[:, dc * 512 + mc * 128 : dc * 512 + (mc + 1) * 128],
                            fT_sb[:, dc * TC : (dc + 1) * TC],
                            start=(dc == 0),
                            stop=(dc == 3),
                        )
                    nc.scalar.activation(
                        tfT[mc][:], ps[:], TANH, bias=bp_sb[:, mc : mc + 1]
                    )
                for mc in range(4):
                    ps = pp.tile([128, TC], FP32, tag="fe", bufs=4)
                    for dc in range(4):
                        nc.tensor.matmul(
                            ps[0:128, 0:U],
                            Wd_sb[:, dc * 512 + mc * 128 : dc * 512 + (mc + 1) * 128],
                            gT_sb[:, dc * U : (dc + 1) * U],
                            start=(dc == 0),
                            stop=(dc == 3),
                        )
                    # tanh into the low u-block; DVE (idle here) duplicates
                    # it so the C GEMM replicates rows
                    nc.scalar.activation(
                        tgT2[mc][:, 0:U], ps[0:128, 0:U], TANH,
                        bias=bp_sb[:, 4 + mc : 5 + mc],
                    )
                    nc.vector.tensor_copy(tgT2[mc][:, U:128], tgT2[mc][:, 0:U])

                # A path first (WjT arrives in parallel on the scalar ring)
                for vh in range(2):
                    vs = slice(vh * 512, (vh + 1) * 512)
                    ps = pp.tile([128, 512], FP32, tag="big", bufs=4)
                    for mc in range(4):
                        nc.tensor.matmul(
                            ps[:],
                            tfT[mc][:],
                            WjT_sb[:, mc * V + vh * 512 : mc * V + (vh + 1) * 512],
                            start=(mc == 0),
                            stop=(mc == 3),
                        )
                    if vh == 0:
                        nc.vector.tensor_copy(A_bf[:, vs], ps[:])
                    else:
                        nc.scalar.copy(A_bf[:, vs], ps[:])

                # hoisted first-tile select-MMs (vh0): only need A_bf vh0,
                # so they run while WjB is still streaming in
                psE = [
                    pp.tile([128, 512], FP32, tag="big", bufs=4, name=f"psE{k}")
                    for k in range(2)
                ]
                obE = [
                    wp.tile([128, V], FP32, tag=f"obE{k}", name=f"obE{k}")
                    for k in range(2)
                ]
                for k in range(2):
                    nc.tensor.matmul(
                        psE[k][:],
                        sel32[0:32, k * 128 : (k + 1) * 128],
                        A_bf[0:32, 0:512],
                        start=True,
                        stop=True,
                        tile_position=(0, 0),
                    )

                # C path: emits the u-replicated Crep directly (+bj)
                for vh in range(2):
                    vs = slice(vh * 512, (vh + 1) * 512)
                    ps = pp.tile([128, 512], FP32, tag="big", bufs=4)
                    # bias MM first: ones1/bj are tiny early loads, so this
                    # runs long before WjB arrives
                    nc.tensor.matmul(
                        ps[:], ones1[:], bj_sb[:, vs], start=True, stop=False
                    )
                    for mc in range(4):
                        nc.tensor.matmul(
                            ps[:],
                            tgT2[mc][:],
                            WjB_sb[:, mc * V + vh * 512 : mc * V + (vh + 1) * 512],
                            start=False,
                            stop=(mc == 3),
                        )
                    if vh == 0:
                        nc.vector.tensor_copy(Crep[:, vs], ps[:])
                    else:
                        nc.scalar.copy(Crep[:, vs], ps[:])

                # complete tiles k=0,1: vh0 add+store right after Crep-vh0,
                # then the vh1 half
                for k in range(2):
                    nc.vector.tensor_add(
                        obE[k][:, 0:512], psE[k][:], Crep[:, 0:512]
                    )
                    nc.sync.dma_start(
                        out_d[k * 128 : (k + 1) * 128, 0:512], obE[k][:, 0:512]
                    )
                for k in range(2):
                    psv = pp.tile([128, 512], FP32, tag="big", bufs=4, name=f"psv{k}")
                    nc.tensor.matmul(
                        psv[:],
                        sel32[0:32, k * 128 : (k + 1) * 128],
                        A_bf[0:32, 512:1024],
                        start=True,
                        stop=True,
                        tile_position=(0, 0),
                    )
                    nc.vector.tensor_add(
                        obE[k][:, 512:1024], psv[:], Crep[:, 512:1024]
                    )
                    nc.sync.dma_start(
                        out_d[k * 128 : (k + 1) * 128, 512:1024],
                        obE[k][:, 512:1024],
                    )

            # ---- main loop: remaining 62 output tiles of [128, 1024] ----
            with (
                tc.tile_pool(name="po", bufs=4, space="PSUM") as po,
                tc.tile_pool(name="ob", bufs=8) as ob,
            ):
                for k in range(2, 64):
                    q, i = k // 16, k % 16
                    rs = slice(32 * q, 32 * q + 32)
                    psO = po.tile([128, V], FP32, tag="psO")
                    out_sb = ob.tile([128, V], FP32, tag="out")
                    for vh in range(2):
                        vs = slice(vh * 512, (vh + 1) * 512)
                        nc.tensor.matmul(
                            psO[:, vs],
                            sel32[rs, i * 128 : (i + 1) * 128],
                            A_bf[rs, vs],
                            start=True,
                            stop=True,
                            tile_position=(32 * q, 0),
                        )
                    # single full-width DVE add does C + the PSUM->SBUF move
                    nc.vector.tensor_add(out_sb[:], psO[:], Crep[:])
                    nc.sync.dma_start(
                        out_d[k * 128 : (k + 1) * 128, :], out_sb[:]
                    )

    nc.compile()
    return nc


def kernel(f, g, We, be, Wd, bd, Wj, bj):
    if "nc" not in _cache:
        _cache["nc"] = _build_nc()
    nc = _cache["nc"]

    bf = lambda x: np.ascontiguousarray(
        np.asarray(x, dtype=np.float32).astype(NPBF16)
    )
    f_bf, g_bf = bf(f), bf(g)
    be32 = np.asarray(be, np.float32).reshape(4, 128).T
    bd32 = np.asarray(bd, np.float32).reshape(4, 128).T
    bias_pack = np.ascontiguousarray(
        np.concatenate([be32, bd32], axis=1), dtype=np.float32
    )
    shared = {
        "We": bf(We), "Wd": bf(Wd), "Wj": bf(Wj),
        "bias_pack": bias_pack, "bj": bf(bj).reshape(1, V),
    }
    in_maps = []
    for c in range(NCORES):
        b, th = c // 2, c % 2
        in_maps.append(
            {
                "fT_c": np.ascontiguousarray(f_bf[b, th * TC : (th + 1) * TC, :].T),
                "gT_c": np.ascontiguousarray(g_bf[b].T),
                **shared,
            }
        )
    res = run_bass_kernel_spmd(nc, in_maps, list(range(NCORES)))
    kernel._last_results = res

    out = np.empty((B, T, U, V), np.float32)
    for c in range(NCORES):
        b, th = c // 2, c % 2
        out[b, th * TC : (th + 1) * TC] = res.results[c]["out"].reshape(TC, U, V)
    return out
